# revision 27
# baseline (speedup 1.0000x reference)
"""Multi-head causal attention with RoPE on 8 TRN2 NeuronCores.

Tensor-parallel over heads: core c computes heads (2c, 2c+1). Single
fused region keeps the PE gapless (full 2.4 GHz p-state needs ~3us of
continuous execution):

  x-chunk pairs (1024 tokens) are projected to Q^T/K^T (RoPE) and V —
  all-bf16 operands, Q/K/V resident in SBUF — and causal-attention
  sections (batch, head, 512-query chunk) are emitted as soon as their
  chunks exist, so attention's act-engine handoffs are filled with
  projection matmuls and vice versa. Per-(batch, head) half-AllToAlls
  (context head-shard -> token-shard) issue mid-stream as each head's
  context completes and hide under later compute. The tail applies the
  full Wo per batch to the gathered 256-token slices; Wo-b0 and the
  even-head half of Wo-b1 (staged to SBUF) fill the last collective's
  latency, leaving only the odd half plus a DVE add after it.

Softmax: scores^T = K^T_blk^T @ Q^T per 128-key block, exp on the Act
engine (no max-subtraction; scores are O(1)), denominators via a
ones-matmul over the partition axis, reciprocal via the fast DVE
approximation, partition-broadcast on gpsimd — the whole softmax tail
stays off the PE.

Host does layout prep (x transpose + bf16 cast, RoPE tables, causal
mask tiles) and final unshard (interleave per-core token slices).
"""
import ml_dtypes
import numpy as np

import concourse.bass as bass  # noqa: F401  (engine namespaces live on nc)
import concourse.mybir as mybir
import concourse.tile as tile
from concourse import bacc
from concourse import bass_utils

B, T, DM, H, D = 2, 2048, 2048, 16, 128
NCORES = 8
HPC = H // NCORES        # heads per core
DLOC = HPC * D           # local head width (256)
BT = B * T               # 4096 token rows
P = 128
TCH = 512                # query chunk
XCH = 1024               # x-chunk pair width
NKB = DM // P            # 16 contraction blocks
NTB = T // P             # 16 token blocks per batch
NBCH = T // TCH          # 4 query chunks per batch
TSL = T // NCORES        # 256-token output slice per core per batch
SCALE = 1.0 / float(np.sqrt(D))
F32 = mybir.dt.float32
F32R = mybir.dt.float32r
BF16 = mybir.dt.bfloat16
MUL = mybir.AluOpType.mult
ADD = mybir.AluOpType.add

_nc_cache = None


def _build():
    nc = bacc.Bacc("TRN2", target_bir_lowering=False, debug=False,
                   num_devices=NCORES)
    xt = nc.dram_tensor("xt", [DM, BT], BF16, kind="ExternalInput")
    wq = nc.dram_tensor("wq", [DM, DLOC], BF16, kind="ExternalInput")
    wk = nc.dram_tensor("wk", [DM, DLOC], BF16, kind="ExternalInput")
    wv = nc.dram_tensor("wv", [DM, DLOC], BF16, kind="ExternalInput")
    wo = nc.dram_tensor("wo", [DM, DM], BF16, kind="ExternalInput")
    cf = nc.dram_tensor("cf", [P, T], F32, kind="ExternalInput")
    sf = nc.dram_tensor("sf", [P, T], F32, kind="ExternalInput")
    cm = nc.dram_tensor("cm", [P, 4 * TCH], BF16, kind="ExternalInput")
    onec = nc.dram_tensor("onec", [P, 1], BF16, kind="ExternalInput")
    # out^T slice: [out_cols, b0 slice | b1 slice]
    outT = nc.dram_tensor("out", [DM, B * TSL], F32, kind="ExternalOutput")

    with tile.TileContext(nc) as tc:
        with tc.tile_pool(name="dram", bufs=1, space="DRAM") as dpool, \
             tc.tile_pool(name="const", bufs=1) as cpool, \
             tc.tile_pool(name="qkv", bufs=1) as qpool:
            # per-(batch, local-head) A2A halves: each fires as soon as that
            # head's context is complete, so the tail Wo can start on the
            # gathered half while the other half is still in flight
            ctxH_d = [[dpool.tile([NCORES * P, TSL], BF16, name=f"ctxH{b}{hl}")
                       for hl in range(HPC)] for b in range(B)]
            gouth_d = [[dpool.tile([NCORES * P, TSL], BF16, name=f"gouth{b}{hl}")
                        for hl in range(HPC)] for b in range(B)]
            bar_in = dpool.tile([8, 4], F32)
            bar_out = dpool.tile([64, 4], F32, addr_space="Shared")

            qT_sb = [qpool.tile([P, HPC, T], BF16, name=f"qT{b}") for b in range(B)]
            kT_sb = [qpool.tile([P, HPC, T], BF16, name=f"kT{b}") for b in range(B)]
            v_sb = [qpool.tile([P, NTB, DLOC], BF16, name=f"v{b}") for b in range(B)]

            cm_s = cpool.tile([P, 4 * TCH], BF16)
            onec_s = cpool.tile([P, 1], BF16)

            # start-skew absorber: cores align here while projections run
            nc.sync.dma_start(bar_in[:], cf.ap()[0:8, 0:4])
            nc.gpsimd.collective_compute(
                "AllGather", mybir.AluOpType.bypass,
                replica_groups=[list(range(NCORES))],
                ins=[bar_in[:].opt()], outs=[bar_out[:].opt()])

            # ---------- fused projections + attention ----------
            with tc.tile_pool(name="p2", bufs=2) as pool2, \
                 tc.tile_pool(name="p2t", bufs=18) as ppool, \
                 tc.tile_pool(name="ps_s", bufs=2, space="PSUM") as ps_sp, \
                 tc.tile_pool(name="ps_acc", bufs=1, space="PSUM") as ps_accp, \
                 tc.tile_pool(name="ps_sum", bufs=1, space="PSUM") as ps_sump:

                def attn_section(b, hl, cq):
                    """One (batch, head, 512-query-chunk) causal-attention
                    section; needs x-chunks <= cq of batch b projected.
                    Diagonal key blocks drop their fully-masked left columns
                    (widths 512/384/256/128)."""
                    nblk = 4 * cq + 4
                    q0 = cq * TCH
                    ps_ctx = ps_accp.tile([P, TCH], F32, tag="ctx")
                    ps_sum = ps_sump.tile([1, TCH], F32, tag="sum")
                    pTs = []
                    for j in range(nblk):
                        vmask = j - 4 * cq
                        off = vmask * P if vmask > 0 else 0
                        sk = (off > 0)
                        ps_sc = ps_sp.tile([P, TCH], F32, tag="s")
                        nc.tensor.matmul(
                            ps_sc[:, off:], kT_sb[b][:, hl, j * P:(j + 1) * P],
                            qT_sb[b][:, hl, q0 + off:q0 + TCH],
                            start=True, stop=True)
                        pT = ppool.tile([P, TCH], BF16, tag="pT")
                        nc.scalar.activation(
                            pT[:, off:], ps_sc[:, off:],
                            mybir.ActivationFunctionType.Exp, scale=SCALE)
                        if vmask >= 0:
                            nc.vector.tensor_tensor(
                                pT[:, off:], pT[:, off:],
                                cm_s[:, vmask * TCH + off:(vmask + 1) * TCH],
                                MUL)
                        nc.tensor.matmul(
                            ps_ctx[:, off:], v_sb[b][:, j, hl * D:(hl + 1) * D],
                            pT[:, off:], start=(j == 0), stop=(j == nblk - 1),
                            skip_group_check=sk)
                        pTs.append((pT, off))
                    # ones-matmuls grouped: the shared stationary's LDWEIGHTS
                    # pipelines under the previous ones-matmul's stream
                    # instead of serializing between ctx and ones every block
                    for j, (pT, off) in enumerate(pTs):
                        nc.tensor.matmul(
                            ps_sum[:, off:], onec_s[:], pT[:, off:],
                            start=(j == 0), stop=(j == nblk - 1),
                            skip_group_check=(off > 0))
                    # softmax tail entirely off the PE
                    rs = pool2.tile([1, TCH], F32, tag="rs")
                    nc.vector.reciprocal_approx_fast(rs[:], ps_sum[:])
                    bc_s = pool2.tile([P, TCH], F32, tag="bc_s")
                    nc.gpsimd.partition_broadcast(bc_s[:], rs[:])
                    ctx_s = pool2.tile([P, TCH], BF16, tag="ctx")
                    nc.vector.tensor_tensor(ctx_s[:], ps_ctx[:], bc_s[:], MUL)
                    nc.sync.dma_start(
                        ctxH_d[b][hl]
                        .rearrange("(r p) n -> p r n", p=P)[:, 2 * cq:2 * cq + 2],
                        ctx_s.rearrange("p (r n) -> p r n", r=2))

                projpools = tc.tile_pool(name="p1w", bufs=1), \
                    tc.tile_pool(name="p1cf", bufs=1), \
                    tc.tile_pool(name="p1x", bufs=2), \
                    tc.tile_pool(name="p1", bufs=2), \
                    tc.tile_pool(name="ps1", bufs=3, space="PSUM"), \
                    tc.tile_pool(name="ps1v", bufs=1, space="PSUM")
                wpool = projpools[0].__enter__()
                cfpool = projpools[1].__enter__()
                xpool = projpools[2].__enter__()
                pool = projpools[3].__enter__()
                ps1 = projpools[4].__enter__()
                ps1v = projpools[5].__enter__()
                wq_s = wpool.tile([P, NKB, DLOC], BF16)
                wk_s = wpool.tile([P, NKB, DLOC], BF16)
                wv_s = wpool.tile([P, NKB, DLOC], BF16)
                cf_s = cfpool.tile([P, T], F32)
                sf_s = cfpool.tile([P, T], F32)

                for ip in range(BT // XCH):     # 4 chunk-pairs
                    bb, icp = ip // 2, ip % 2
                    xt_t = xpool.tile([P, NKB, XCH], BF16, tag="xt")
                    for kb in range(NKB):
                        if ip == 0:
                            nc.sync.dma_start(wq_s[:, kb],
                                              wq.ap()[kb * P:(kb + 1) * P, :])
                            nc.sync.dma_start(wk_s[:, kb],
                                              wk.ap()[kb * P:(kb + 1) * P, :])
                        nc.sync.dma_start(
                            xt_t[:, kb],
                            xt.ap()[kb * P:(kb + 1) * P, ip * XCH:(ip + 1) * XCH])
                        if ip == 0 and kb == 3:
                            nc.sync.dma_start(cf_s[:, 0:XCH], cf.ap()[:, 0:XCH])
                            nc.sync.dma_start(sf_s[:, 0:XCH], sf.ap()[:, 0:XCH])
                    if ip == 0:
                        nc.sync.dma_start(cf_s[:, XCH:], cf.ap()[:, XCH:])
                        nc.sync.dma_start(sf_s[:, XCH:], sf.ap()[:, XCH:])
                        for kb in range(NKB):
                            nc.sync.dma_start(wv_s[:, kb],
                                              wv.ap()[kb * P:(kb + 1) * P, :])
                        nc.sync.dma_start(cm_s[:], cm.ap())
                        nc.sync.dma_start(onec_s[:], onec.ap())
                    # two query-chunk columns per stationary pass
                    for w_s, dst in ((wq_s, qT_sb), (wk_s, kT_sb)):
                        for m in range(HPC):
                            psa = ps1.tile([P, TCH], F32, tag="qk")
                            psb = ps1.tile([P, TCH], F32, tag="qk")
                            for kb in range(NKB):
                                st, sp = (kb == 0), (kb == NKB - 1)
                                w_blk = w_s[:, kb, m * P:(m + 1) * P]
                                nc.tensor.matmul(psa[:], w_blk,
                                                 xt_t[:, kb, 0:TCH],
                                                 start=st, stop=sp)
                                nc.tensor.matmul(psb[:], w_blk,
                                                 xt_t[:, kb, TCH:XCH],
                                                 start=st, stop=sp)
                            for half, ps in ((0, psa), (1, psb)):
                                ic = 2 * icp + half
                                c0 = ic * TCH
                                cs = cf_s[:, c0:c0 + TCH]
                                sn = sf_s[:, c0:c0 + TCH]
                                tmp = pool.tile([P, TCH], F32, tag="tmp")
                                tmp2 = pool.tile([P, TCH], F32, tag="tmp2")
                                nc.vector.tensor_tensor(tmp[0:64], ps[64:128],
                                                        sn[0:64], MUL)
                                nc.vector.tensor_tensor(tmp[64:128], ps[0:64],
                                                        sn[64:128], MUL)
                                nc.vector.tensor_tensor(tmp2[:], ps[:], cs, MUL)
                                nc.vector.tensor_tensor(
                                    dst[bb][:, m, c0:c0 + TCH],
                                    tmp2[:], tmp[:], ADD)
                    for tb in range(XCH // P):
                        psv = ps1v.tile([P, DLOC], F32, tag="v")
                        for kb in range(NKB):
                            nc.tensor.matmul(
                                psv[:], xt_t[:, kb, tb * P:(tb + 1) * P],
                                wv_s[:, kb],
                                start=(kb == 0), stop=(kb == NKB - 1))
                        nc.scalar.activation(
                            v_sb[bb][:, icp * (XCH // P) + tb, :], psv[:],
                            mybir.ActivationFunctionType.Copy)
                    # attention sections whose query chunks now exist; the
                    # last pair's sections are deferred past the projection
                    # pools so they interleave with Wo-b0 instead
                    if ip < 3:
                        if icp == 0:
                            for cq in (0, 1):
                                for hl in range(HPC):
                                    attn_section(bb, hl, cq)
                        else:
                            for hl in range(HPC):
                                for cq in (2, 3):
                                    attn_section(bb, hl, cq)
                                nc.gpsimd.collective_compute(
                                    "AllToAll", mybir.AluOpType.bypass,
                                    replica_groups=[list(range(NCORES))],
                                    ins=[ctxH_d[0][hl][:].opt()],
                                    outs=[gouth_d[0][hl][:].opt()])

                for p in reversed(projpools):
                    p.__exit__(None, None, None)

                # ---------- tail: last sections + output projection ----------
                # Wo-b0 matmuls become the PE filler for the deferred
                # sections' act-engine stalls, and A2A-b1 hides under them.
                with tc.tile_pool(name="p3w", bufs=1) as wpool3, \
                     tc.tile_pool(name="p3", bufs=2) as pool3, \
                     tc.tile_pool(name="ps3", bufs=4, space="PSUM") as ps3:
                    wo_s = wpool3.tile([P, NKB, DM], BF16)
                    g_t = [wpool3.tile([P, NKB, TSL], BF16, name=f"g{b}")
                           for b in range(B)]
                    # gathered-context loads ride the scalar hwdge queue so
                    # the parked A2A waits never block ctx scatter writes;
                    # gouth half block r holds global head 2r+hl -> kb 2r+hl
                    for hl in range(HPC):
                        for r in range(NCORES):
                            nc.scalar.dma_start(
                                g_t[0][:, 2 * r + hl],
                                gouth_d[0][hl]
                                .rearrange("(r p) n -> r p n", p=P)[r])
                    for half in range(2):
                        for kb in range(NKB):
                            nc.sync.dma_start(
                                wo_s[:, kb, half * XCH:(half + 1) * XCH],
                                wo.ap()[kb * P:(kb + 1) * P,
                                        half * XCH:(half + 1) * XCH])
                    # cq2 first: its queries were projected earlier, so the
                    # first deferred section never waits on the last RoPE;
                    # each head-half's AllToAll fires when its context is done
                    for hl in range(HPC):
                        for cq in (2, 3):
                            attn_section(1, hl, cq)
                        nc.gpsimd.collective_compute(
                            "AllToAll", mybir.AluOpType.bypass,
                            replica_groups=[list(range(NCORES))],
                            ins=[ctxH_d[1][hl][:].opt()],
                            outs=[gouth_d[1][hl][:].opt()])
                        for r in range(NCORES):
                            nc.scalar.dma_start(
                                g_t[1][:, 2 * r + hl],
                                gouth_d[1][hl]
                                .rearrange("(r p) n -> r p n", p=P)[r])
                    # Wo-b0: full 16-kb groups (g0 gathered long ago).
                    for m in range(DM // P):
                        pso = ps3.tile([P, TSL], F32, tag="o")
                        for kb in range(NKB):
                            nc.tensor.matmul(
                                pso[:], wo_s[:, kb, m * P:(m + 1) * P],
                                g_t[0][:, kb],
                                start=(kb == 0), stop=(kb == NKB - 1))
                        o_s = pool3.tile([P, TSL], F32, tag="o_s")
                        nc.scalar.activation(
                            o_s[:], pso[:], mybir.ActivationFunctionType.Copy)
                        nc.sync.dma_start(
                            outT.ap()[m * P:(m + 1) * P, 0:TSL], o_s[:])
                    # Wo-b1 split by head half: ALL even-kb partial sums run
                    # before the last AllToAll lands (gap filler, staged to
                    # SBUF); after it only the odd half + a DVE add remain.
                    evnB = wpool3.tile([P, NKB, TSL], F32, name="evnB")
                    for m in range(DM // P):
                        psA = ps3.tile([P, TSL], F32, tag="o")
                        for i in range(NCORES):
                            nc.tensor.matmul(
                                psA[:], wo_s[:, 2 * i, m * P:(m + 1) * P],
                                g_t[1][:, 2 * i],
                                start=(i == 0), stop=(i == NCORES - 1))
                        nc.scalar.activation(
                            evnB[:, m], psA[:],
                            mybir.ActivationFunctionType.Copy)
                    for m in range(DM // P):
                        psB = ps3.tile([P, TSL], F32, tag="o")
                        for i in range(NCORES):
                            nc.tensor.matmul(
                                psB[:], wo_s[:, 2 * i + 1, m * P:(m + 1) * P],
                                g_t[1][:, 2 * i + 1],
                                start=(i == 0), stop=(i == NCORES - 1))
                        o_s = pool3.tile([P, TSL], F32, tag="o_s")
                        nc.vector.tensor_tensor(o_s[:], psB[:], evnB[:, m], ADD)
                        nc.sync.dma_start(
                            outT.ap()[m * P:(m + 1) * P, TSL:2 * TSL], o_s[:])

    nc.compile()
    return nc


def _prep_inputs(x, cos, sin, Wq, Wk, Wv, Wo):
    x = np.asarray(x, dtype=np.float32)
    cos = np.asarray(cos, dtype=np.float32)
    sin = np.asarray(sin, dtype=np.float32)
    xt = np.ascontiguousarray(x.reshape(BT, DM).T).astype(ml_dtypes.bfloat16)
    cf = np.empty((P, T), np.float32)
    cf[:64] = cos.T
    cf[64:] = cos.T
    sf = np.empty((P, T), np.float32)
    sf[:64] = -sin.T
    sf[64:] = sin.T
    qq = np.arange(TCH, dtype=np.int64)[None, :]
    rr = np.arange(P, dtype=np.int64)[:, None]
    cm = np.concatenate(
        [(qq >= v * P + rr).astype(np.float32) for v in range(TCH // P)],
        axis=1).astype(ml_dtypes.bfloat16)
    onec = np.ones((P, 1), np.float32).astype(ml_dtypes.bfloat16)
    wo_full = np.ascontiguousarray(np.asarray(Wo, np.float32)).astype(ml_dtypes.bfloat16)
    in_maps = []
    for c in range(NCORES):
        sl = slice(c * DLOC, (c + 1) * DLOC)
        in_maps.append({
            "xt": xt, "cf": cf, "sf": sf, "cm": cm, "onec": onec,
            "wq": np.ascontiguousarray(
                np.asarray(Wq, np.float32)[:, sl]).astype(ml_dtypes.bfloat16),
            "wk": np.ascontiguousarray(
                np.asarray(Wk, np.float32)[:, sl]).astype(ml_dtypes.bfloat16),
            "wv": np.ascontiguousarray(
                np.asarray(Wv, np.float32)[:, sl]).astype(ml_dtypes.bfloat16),
            "wo": wo_full,
        })
    return in_maps


def run(x, mask, cos, sin, Wq, Wk, Wv, Wo, trace=False, trace_cores=None):
    global _nc_cache
    if _nc_cache is None:
        _nc_cache = _build()
    in_maps = _prep_inputs(x, cos, sin, Wq, Wk, Wv, Wo)
    kwargs = {"trace_cores": trace_cores} if trace_cores else {}
    res = bass_utils.run_bass_kernel_spmd(
        _nc_cache, in_maps, core_ids=list(range(NCORES)), trace=trace, **kwargs)
    out = np.empty((B, T, DM), np.float32)
    for c in range(NCORES):
        o = res.results[c]["out"]  # [DM, B*TSL]
        for b in range(B):
            out[b, c * TSL:(c + 1) * TSL, :] = o[:, b * TSL:(b + 1) * TSL].T
    return out, res


def kernel(x, mask, cos, sin, Wq, Wk, Wv, Wo):
    out, _ = run(x, mask, cos, sin, Wq, Wk, Wv, Wo, trace=False)
    return out
